# revision 27
# baseline (speedup 1.0000x reference)
"""Trainium2 Bass kernel for a dense causal-attention transformer block.

Reference computation (fp32, B=2, S=2048, D=2048, H=16, HD=128):
    qkv = x @ Wqkv ; q,k,v split per head
    scores = (q @ k^T) * HD**-0.5, causal mask, softmax
    o = softmax(scores) @ v ; out = o @ Wo

Sharding: tensor-parallel over heads (4 groups of 4 heads) x data-parallel
over batch (2) = 8 cores. Each core computes a partial output projection
(its 512 o-channels x Wo rows); the host sums the 4 partials per batch.

Device layout tricks:
  - All matmul inputs are bf16 (4x faster PE than fp32); PSUM accum fp32.
  - qT/kT are produced channels-on-partitions so score tiles come out
    TRANSPOSED [keys=128, queries=512]; softmax sum is then a matmul with
    an all-ones lhsT (no cross-partition reduce, no transposes anywhere).
  - No max-subtraction in softmax: scores ~ N(0,1), exp is safe in fp32,
    and masked entries are multiplied by 0 after exp.
  - HD**-0.5 scaling folded into Wq on the host.
"""

import numpy as np
import ml_dtypes

BF16 = ml_dtypes.bfloat16

B = 2
S = 2048
D = 2048
H = 16
HD = 128
P = 128
G = 4            # TP groups (heads per group = 4)
NH = H // G      # heads per core = 4
CH = NH * HD     # o-channels per core = 512
NJ = S // 512    # 4 S-chunks of 512
KK = D // P      # 16 contraction tiles
ST = S // P      # 16 sequence row-tiles

_progs = {}

# normalizer strategy: "pe" = per-tile ones-matmuls on PE;
# "pair" = one DVE/GpSimd pairwise-add level, then half as many ones-matmuls
# "quad" = two fold levels, quartering the ones-matmuls for full tiles
# "tree2" = zero-pad diagonal tiles (full-width mask) and fold everything
#           down to 2 ones-matmuls per head-step
SUM_MODE = "tree2"


def _build(repeat=1):
    """Build (once) the single-core Bass/Tile program shared by all 8 cores.

    repeat>1 executes the whole computation that many times inside one NEFF
    (used only for overhead-free timing via T(xN)-T(x1) differencing).
    """
    key = (repeat, SUM_MODE)
    if key in _progs:
        return _progs[key]

    import concourse.tile as tile
    from concourse import bacc, mybir

    f32 = mybir.dt.float32
    bf16 = mybir.dt.bfloat16
    EXP = mybir.ActivationFunctionType.Exp

    nc = bacc.Bacc("TRN2", target_bir_lowering=False, debug=False)

    # DRAM I/O, pre-packed on host so every DMA is contiguous per partition.
    # x:  [p, nj, kk, q]  = xT chunk layout (x[b].T tiled)
    # wq/wk: [p, mi, kk, m] (column-sharded Wqkv, q part prescaled by HD^-.5)
    # wv: [p, kk, n]      (rhs layout)
    # wo: [p, h, ncol, n] (row-sharded Wo)
    # masks: [k, j, q]    binary causal masks for the 4 diagonal positions
    # out: [p, si, col]   partial output (fp32)
    x_d = nc.dram_tensor("x", (P, NJ, KK, 512), bf16, kind="ExternalInput")
    wq_d = nc.dram_tensor("wq", (P, NH, KK, P), bf16, kind="ExternalInput")
    wk_d = nc.dram_tensor("wk", (P, NH, KK, P), bf16, kind="ExternalInput")
    wv_d = nc.dram_tensor("wv", (P, KK, CH), bf16, kind="ExternalInput")
    wo_d = nc.dram_tensor("wo", (P, NH, NJ, 512), bf16, kind="ExternalInput")
    mask_d = nc.dram_tensor("masks", (P, NH, 512), bf16, kind="ExternalInput")
    # partial outputs in bf16 (halves output DMA); host sums them in fp32
    out_d = nc.dram_tensor("out", (P, ST, D), bf16, kind="ExternalOutput")

    with tile.TileContext(nc) as tc:
        with (
            tc.tile_pool(name="persist", bufs=1) as pp,
            tc.tile_pool(name="psumA", bufs=2, space="PSUM") as psA,
            tc.tile_pool(name="psumW", bufs=2, space="PSUM") as psW,
            tc.tile_pool(name="psumB", bufs=1, space="PSUM") as psB,
        ):
            for rep in range(repeat):
                _emit_once(nc, tc, tile, mybir, pp, psA, psW, psB,
                           x_d, wq_d, wk_d, wv_d, wo_d, mask_d, out_d,
                           f32, bf16, EXP, rep)

    nc.compile()
    _progs[key] = nc
    return nc


def _emit_once(nc, tc, tile, mybir, pp, psA, psW, psB,
               x_d, wq_d, wk_d, wv_d, wo_d, mask_d, out_d,
               f32, bf16, EXP, rep):
    r = f"r{rep}_"
    # wq/wk as 4 per-head-group tiles so the first matmul group only
    # depends on a 0.5MB DMA, not the whole weight
    wq_t = [pp.tile([P, KK, P], bf16, name=f"{r}wq{mi}", tag=f"wq{mi}")
            for mi in range(NH)]
    wk_t = [pp.tile([P, KK, P], bf16, name=f"{r}wk{mi}", tag=f"wk{mi}")
            for mi in range(NH)]
    # wv (phase 1) and wo (phase 3) share one 16KB slot
    wv_sb = pp.tile([P, KK, CH], bf16, name=r + "wv_sb", tag="wvwo")
    q_sb = pp.tile([P, NH, S], bf16, name=r + "q_sb", tag="q")
    k_sb = pp.tile([P, NH, S], bf16, name=r + "k_sb", tag="k")
    v_sb = pp.tile([P, ST, CH], bf16, name=r + "v_sb", tag="v")
    o_sb = pp.tile([P, NH, S], bf16, name=r + "o_sb", tag="o")
    mask_sb = pp.tile([P, NH, 512], bf16, name=r + "mask_sb", tag="mask")
    ones_sb = pp.tile([P, P], bf16, name=r + "ones_sb", tag="ones")
    zbias = pp.tile([P, 1], f32, name=r + "zbias", tag="zbias")

    nc.gpsimd.memset(ones_sb[:], 1.0)
    nc.gpsimd.memset(zbias[:], 0.0)

    # ---- Phase 1 + 2 interleaved: QKV projections + attention ----
    ADD = mybir.AluOpType.add
    eng_toggle = [0]

    with (
        tc.tile_pool(name=r + "apool", bufs=18) as apool,
        tc.tile_pool(name=r + "tpool", bufs=14) as tpool,
        tc.tile_pool(name=r + "rpool", bufs=3) as rpool,
    ):
        def qkv_thunks(nj, xc):
            """6 thunks, one per 2-bank accumulation group: q, k pairs of
            head-tiles then v pairs of seq-tiles. qT/kT land channel-major
            (per head = 128 partitions); a single ACT copy moves both banks
            out."""
            def qk(w_t, dst, dn, mi0):
                pw = psW.tile([P, 1024], f32,
                              name=f"{r}{dn}{nj}_{mi0}", tag="accW")
                for half in (0, 1):
                    mi = mi0 + half
                    for kk in range(KK):
                        nc.tensor.matmul(
                            pw[:, half * 512:(half + 1) * 512],
                            w_t[mi][:, kk, :], xc[:, kk, :],
                            start=(kk == 0), stop=(kk == KK - 1))
                nc.scalar.copy(
                    out=dst[:, mi0:mi0 + 2, nj * 512:(nj + 1) * 512],
                    in_=pw[:].rearrange("p (a b) -> p a b", a=2))

            def v(si0):
                # v: [S, CH] row-major (keys on partitions), same pairing
                pw = psW.tile([P, 1024], f32,
                              name=f"{r}v{nj}_{si0}", tag="accW")
                for half in (0, 1):
                    si = si0 + half
                    for kk in range(KK):
                        nc.tensor.matmul(
                            pw[:, half * 512:(half + 1) * 512],
                            xc[:, kk, si * P:(si + 1) * P],
                            wv_sb[:, kk, :],
                            start=(kk == 0), stop=(kk == KK - 1))
                nc.scalar.copy(
                    out=v_sb[:, 4 * nj + si0:4 * nj + si0 + 2, :],
                    in_=pw[:].rearrange("p (a b) -> p a b", a=2))

            import functools
            return [functools.partial(qk, wq_t, q_sb, "q", 0),
                    functools.partial(qk, wq_t, q_sb, "q", 2),
                    functools.partial(qk, wk_t, k_sb, "k", 0),
                    functools.partial(qk, wk_t, k_sb, "k", 2),
                    functools.partial(v, 0),
                    functools.partial(v, 2)]

        def emit_A(qc, h, use_psw=False, filler=None):
            """scoresT [keys=128, queries=512]. Woven into QKV (use_psw
            False): single-bank tiles from the psA ring, which is free
            until the projection, so QKV keeps the 2-slot psW ring to
            itself. Woven into the projection (use_psw True): two key
            tiles per 2-bank psW tile — psW is the free ring there.
            `filler()` is called between key-tile pairs to slot in other
            PE work while the pair's exp drains its PSUM slot."""
            qs, qe = qc * 512, (qc + 1) * 512
            ktmax = 4 * qc + 4
            a_slices = []
            for kt0 in range(0, ktmax, 2):
                if filler is not None and kt0 > 0:
                    filler()
                a2 = apool.tile([P, 1024], bf16,
                                name=f"{r}a{qc}_{h}_{kt0}", tag="a")
                pw2 = (psW.tile([P, 1024], f32,
                                name=f"{r}st{qc}_{h}_{kt0}", tag="accW")
                       if use_psw else None)
                for j2 in (0, 1):
                    kt = kt0 + j2
                    # diagonal tiles: queries < 128j are fully masked —
                    # compute, exp, mask and consume only visible columns
                    off = _diag_off(qc, kt)
                    if use_psw:
                        pw = pw2[:, j2 * 512:(j2 + 1) * 512]
                    else:
                        pw = psA.tile([P, 512], f32,
                                      name=f"{r}st{qc}_{h}_{kt}",
                                      tag="accA")[:]
                    nc.tensor.matmul(
                        pw[:, off:],
                        k_sb[:, h, kt * P:(kt + 1) * P],
                        q_sb[:, h, qs + off:qe], start=True, stop=True)
                    nc.scalar.activation(
                        a2[:, j2 * 512 + off:(j2 + 1) * 512],
                        pw[:, off:], EXP, bias=zbias[:])
                    sl = a2[:, j2 * 512:(j2 + 1) * 512]
                    if kt >= 4 * qc:  # diagonal tile: causal 0/1 mask
                        if SUM_MODE == "tree2" and off > 0:
                            # zero the fully-masked [0:off] region the exp
                            # never writes, so this tile folds like a full
                            # one in the normalizer tree
                            nc.gpsimd.memset(sl[:, :off], 0.0)
                        nc.vector.tensor_mul(
                            out=sl[:, off:], in0=sl[:, off:],
                            in1=mask_sb[:, kt - 4 * qc, off:])
                    a_slices.append(sl)
            return a_slices

        def emit_B(qc, h, a_slices):
            """AV accumulation + normalizer + divide for one head."""
            qs, qe = qc * 512, (qc + 1) * 512
            ktmax = 4 * qc + 4
            po = psB.tile([P, 512], f32, name=f"{r}po{qc}_{h}", tag="po")
            for kt in range(ktmax):
                # same column restriction as the scores; kt=0 is always a
                # full-width write, so every po column is initialized by the
                # start=True matmul
                off = _diag_off(qc, kt)
                nc.tensor.matmul(
                    po[:, off:], v_sb[:, kt, h * HD:(h + 1) * HD],
                    a_slices[kt][:, off:],
                    start=(kt == 0), stop=(kt == ktmax - 1))
            # normalizer: column sums of a over all key tiles, replicated
            # to all partitions by the all-ones lhsT
            pn = psB.tile([P, 512], f32, name=f"{r}pn{qc}_{h}", tag="pn")
            # full tiles: pairwise-add on DVE/GpSimd halves the PE
            # sum-matmuls; diagonal tiles go in individually, restricted to
            # their visible columns. (off, rhs) list: full-width entry first
            # so the start=True matmul initializes every pn column.
            def fold(lvl, nfold, stop_at=1):
                for fl in range(nfold):
                    if len(lvl) <= stop_at:
                        break
                    nxt = []
                    for i in range(0, len(lvl) - 1, 2):
                        t = tpool.tile([P, 512], bf16,
                                       name=f"{r}ts{qc}_{h}_{fl}_{i}",
                                       tag="tsum")
                        eng = (nc.vector if eng_toggle[0] % 2 == 0
                               else nc.gpsimd)
                        eng_toggle[0] += 1
                        eng.tensor_tensor(t[:], lvl[i], lvl[i + 1], ADD)
                        nxt.append(t[:])
                    if len(lvl) % 2:
                        nxt.append(lvl[-1])
                    lvl = nxt
                return lvl

            sum_rhs = []
            if SUM_MODE == "tree2":
                # every tile is full-width (diagonals were zero-padded by
                # the full mask): fold all of them down to two
                sum_rhs = [(0, t) for t in
                           fold(list(a_slices), 8, stop_at=2)]
            else:
                full = [a_slices[kt] for kt in range(ktmax)
                        if _diag_off(qc, kt) == 0 and kt < 4 * qc]
                diag = [(kt, _diag_off(qc, kt)) for kt in range(ktmax)
                        if kt >= 4 * qc]
                if SUM_MODE in ("pair", "quad") and len(full) >= 2:
                    nfold = 1 if SUM_MODE == "pair" else 2
                    sum_rhs = [(0, t) for t in fold(list(full), nfold)]
                else:
                    sum_rhs = [(0, s) for s in full]
                sum_rhs += [(off, a_slices[kt][:, off:])
                            for kt, off in diag]
            for i, (off, t) in enumerate(sum_rhs):
                nc.tensor.matmul(pn[:, off:], ones_sb[:], t,
                                 start=(i == 0),
                                 stop=(i == len(sum_rhs) - 1))
            rec = rpool.tile([P, 512], f32, name=f"{r}rc{qc}_{h}",
                             tag="rec")
            nc.vector.reciprocal_approx_fast(rec[:], pn[:])
            nc.vector.tensor_mul(out=o_sb[:, h, qs:qe],
                                 in0=po[:], in1=rec[:])

        xpool_cm = tc.tile_pool(name=r + "xpool", bufs=2)
        xpool = xpool_cm.__enter__()
        # DMA issue order = arrival order: first x chunk (split in half)
        # and first weight slice land before everything else so PE can
        # start within a few us
        xcs = {0: xpool.tile([P, KK, 512], bf16, name=f"{r}xc0", tag="xc")}
        nc.sync.dma_start(wq_t[0][:, :KK // 2], wq_d[:, 0, :KK // 2])
        nc.sync.dma_start(
            xcs[0][:, :KK // 4], x_d[:, 0, :KK // 4])
        nc.sync.dma_start(wq_t[0][:, KK // 2:], wq_d[:, 0, KK // 2:])
        for qtr in range(1, 4):
            nc.sync.dma_start(
                xcs[0][:, qtr * KK // 4:(qtr + 1) * KK // 4],
                x_d[:, 0, qtr * KK // 4:(qtr + 1) * KK // 4])
        for mi in range(1, NH):
            nc.sync.dma_start(wq_t[mi][:], wq_d[:, mi])
        for mi in range(NH):
            nc.sync.dma_start(wk_t[mi][:], wk_d[:, mi])
        nc.sync.dma_start(wv_sb[:], wv_d[:])
        nc.sync.dma_start(mask_sb[:], mask_d[:])

        # software pipeline: chunk nj's QKV groups woven with chunk nj-1's
        # attention head-steps (their q/k/v landed a full chunk ago, so no
        # copy-latency stalls); emit_B for a step runs one head-step after
        # its emit_A so the exps have time to finish. The last chunk's
        # steps run back-to-back after its QKV.
        pend = [None]

        def attn(qc, h, use_psw=False):
            a = emit_A(qc, h, use_psw)
            if pend[0] is not None:
                emit_B(*pend[0])
            pend[0] = (qc, h, a)

        for nj in range(NJ):
            xc = xcs.get(nj)
            if xc is None:
                xc = xpool.tile([P, KK, 512], bf16, name=f"{r}xc{nj}",
                                tag="xc")
                nc.sync.dma_start(xc[:], x_d[:, nj])
            for i, thunk in enumerate(qkv_thunks(nj, xc)):
                thunk()
                if nj > 0 and 1 <= i <= NH:
                    attn(nj - 1, i - 1)
        xpool_cm.__exit__(None, None, None)

        # ------- Phase 3: last chunk's attention woven with the -------
        # ------- output projection (scores now on the psW ring) -------
        # wo reuses wv's slot (Tile starts the DMA once nj=3's v groups
        # finish; the qc=3 attention ahead of the first units hides it)
        wo_sb = pp.tile([P, NH, NJ, 512], bf16, name=r + "wo_sb",
                        tag="wvwo")
        nc.sync.dma_start(wo_sb[:], wo_d[:])
        with tc.tile_pool(name=r + "ostage", bufs=4) as ostage:
            units = []
            for qc in range(NJ - 1):
                units.extend(_proj_units(nc, psA, ostage, o_sb, wo_sb,
                                         out_d, qc, r))
            it = iter(units)

            def filler():
                u = next(it, None)
                if u is not None:
                    u()

            for h in range(NH):
                a = emit_A(NJ - 1, h, use_psw=True, filler=filler)
                if pend[0] is not None:
                    emit_B(*pend[0])
                pend[0] = (NJ - 1, h, a)
            for u in it:
                u()
            emit_B(*pend[0])
            for u in _proj_units(nc, psA, ostage, o_sb, wo_sb, out_d,
                                 NJ - 1, r):
                u()


def _diag_off(qc, kt):
    """First visible query column (within the 512 chunk) for key tile kt of
    chunk qc; 0 for fully-visible tiles."""
    if kt < 4 * qc:
        return 0
    return 128 * (kt - 4 * qc)


def _proj_units(nc, psA, ostage, o_sb, wo_sb, out_d, qc, r):
    """8 thunks (4 si x 2 column groups) projecting chunk qc; each is
    ~0.9us of PE, sized to slot between attention score pairs."""
    import concourse.mybir as mybir
    f32 = mybir.dt.float32
    bf16 = mybir.dt.bfloat16

    def unit(si, nc0):
        # two column-block groups share one stage tile -> one DMA
        stg = ostage.tile([P, 1024], bf16,
                          name=f"{r}os{si}_{nc0}", tag="os")
        for half in (0, 1):
            ncol = nc0 + half
            acc = psA.tile([P, 512], f32,
                           name=f"{r}pr{si}_{ncol}", tag="accA")
            for h in range(NH):
                nc.tensor.matmul(
                    acc[:], o_sb[:, h, si * P:(si + 1) * P],
                    wo_sb[:, h, ncol, :],
                    start=(h == 0), stop=(h == NH - 1))
            nc.vector.tensor_copy(
                out=stg[:, half * 512:(half + 1) * 512], in_=acc[:])
        nc.sync.dma_start(
            out_d[:, si, nc0 * 512:(nc0 + 2) * 512], stg[:])

    import functools
    return [functools.partial(unit, si, nc0)
            for si in range(4 * qc, 4 * qc + 4) for nc0 in (0, 2)]


def _pack_inputs(x, Wqkv, Wo):
    """Host-side shard + pack into the per-core DMA-friendly layouts.
    Arrays are shared between cores where identical (x per batch, weights
    per TP group, masks global)."""
    scale = np.float32(HD) ** np.float32(-0.5)
    masks = np.zeros((P, NH, 512), dtype=BF16)
    k_idx = np.arange(P)[:, None]
    q_idx = np.arange(512)[None, :]
    for j in range(NH):
        masks[:, j, :] = (P * j + k_idx <= q_idx).astype(BF16)

    xps = []
    for b in range(B):
        xb = np.asarray(x[b], dtype=np.float32)
        # xT packed: [p, nj, kk, q] with xT[128*kk+p, 512*nj+q] = xb[q', d']
        xps.append(np.ascontiguousarray(
            xb.astype(BF16).reshape(NJ, 512, KK, P).transpose(3, 0, 2, 1)))

    wmaps = []
    for g in range(G):
        wq = (np.asarray(Wqkv[:, CH * g:CH * (g + 1)], np.float32) * scale)
        wk = np.asarray(Wqkv[:, D + CH * g:D + CH * (g + 1)], np.float32)
        wv = np.asarray(Wqkv[:, 2 * D + CH * g:2 * D + CH * (g + 1)],
                        np.float32)
        wo = np.asarray(Wo[CH * g:CH * (g + 1), :], np.float32)
        wmaps.append({
            "wq": np.ascontiguousarray(
                wq.astype(BF16).reshape(KK, P, NH, P).transpose(1, 2, 0, 3)),
            "wk": np.ascontiguousarray(
                wk.astype(BF16).reshape(KK, P, NH, P).transpose(1, 2, 0, 3)),
            "wv": np.ascontiguousarray(
                wv.astype(BF16).reshape(KK, P, CH).transpose(1, 0, 2)),
            "wo": np.ascontiguousarray(
                wo.astype(BF16).reshape(NH, P, NJ, 512).transpose(1, 0, 2, 3)),
        })

    return [{"x": xps[c // G], "masks": masks, **wmaps[c % G]}
            for c in range(8)]


def _unpack_outputs(results):
    """Sum the 4 TP partials per batch and restore [B, S, D]."""
    out = np.zeros((B, S, D), dtype=np.float32)
    for c, res in enumerate(results):
        b = c // G
        part = np.asarray(res["out"]).astype(np.float32)   # [p, si, col]
        out[b] += part.transpose(1, 0, 2).reshape(S, D)
    return out


def kernel(x, Wqkv, Wo, _trace=False, _trace_kwargs=None):
    from concourse import bass_utils

    nc = _build()
    in_maps = _pack_inputs(x, Wqkv, Wo)
    res = bass_utils.run_bass_kernel_spmd(
        nc, in_maps, core_ids=list(range(8)), trace=_trace,
        **(_trace_kwargs or {}))
    out = _unpack_outputs(res.results)
    if _trace:
        kernel.last_result = res
    return out

